# revision 33
# baseline (speedup 1.0000x reference)
"""RBF-kernel SVM inference on 8 Trainium2 NeuronCores.

out[m] = sum_n w[n] * exp(-g * ||x[m] - xt[n]||^2) + b
       = exp(-g*||x[m]||^2) * sum_n w[n] * exp(2g*x[m].xt[n] - g*||xt[n]||^2) + b

Sharding: rows of x split across 8 cores (1024 each); x_train / weight
replicated. Per core: a [8192, 1024] kernel slab via fp8 DoubleRow matmuls
(K=512 contraction, n on partitions).

The exp and the n-reduction are split across three engines to balance load:
  - PE: main matmuls (fixed cost) + DoubleRow reduction matmuls for a few
    "PE-red" pairs (fp8 e-planes, weights in the stationary) + a final bf16
    ones-reduction of the DVE accumulator.
  - ACT: exp for most tiles; the per-partition bias carries -g*||xt||^2, plus
    ln(256|w|) for DVE-reduced tiles (weights folded into the exp).
  - DVE: Schraudolph-style exp for some tiles (one tensor_scalar: affine into
    bf16/fp8 bit space with RNE saturating int convert ~= 2^((b-bias)/mant);
    negative saturation yields -0.0 = exp underflow), and accumulation of the
    DVE-reduced pairs' bf16 e-planes into a bf16 accumulator (training points
    pre-sorted by sign(w) so tiles are sign-pure and +/- becomes add/subtract).

Startup: the x slab and lead x_train groups split across the sync HWDGE and
gpsimd SWDGE queues (nothing on the scalar engine, which the exp saturates),
and warmup matmuls on scratch so the PE clock (HAM) is at 2.4 GHz and the
DMA data is resident when the first real matmul lands.
"""

import os
import sys

for _p in ("/opt/trn_rl_repo", "/root/.axon_site/_ro/trn_rl_repo"):
    if os.path.isdir(_p) and _p not in sys.path:
        sys.path.append(_p)

import numpy as np
import ml_dtypes

import concourse.bass as bass
import concourse.mybir as mybir
import concourse.tile as tile
from concourse import bacc
from concourse.bass_utils import run_bass_kernel_spmd

M, N, D = 8192, 8192, 512
NCORES = 8
MC = M // NCORES          # rows of x per core
P = 128
KT = D // P               # K tiles (4)
NT = N // P               # n tiles (64)
NPAIR = NT // 2           # n-tile pairs (32)
MCH = 512                 # PSUM free-dim chunk (one f32 bank)
MJ = MC // MCH            # m chunks (2)

C1_8 = 8.0 / float(np.log(2.0))      # fp8e4m3 bits per ln unit
C2_8 = 56.0                          # fp8e4m3 exponent bias offset
C1_16 = 128.0 / float(np.log(2.0))   # bf16 bits per ln unit
C2_16 = 16256.0                      # bf16 exponent bias offset (127*128)

LAST_RESULTS = None  # BassKernelResults of the most recent run (for test.py)

N_DVE_EXP = 13


def _assignments(tmix_pair):
    # PE-red pairs: the mixed-sign pair + the last two pairs (so the tail
    # doesn't wait on the DVE accumulator)
    pe_red = {NPAIR - 2, NPAIR - 1}
    if tmix_pair is not None:
        pe_red.add(tmix_pair)
    # DVE-exp tiles: alternate with ACT over the first tiles (pipeline fill),
    # one tile of each of the last two pairs (parallel tail), rest spread on
    # even (h=0) positions so the pair's accumulate waits on the ACT-produced
    # h=1 plane instead of stacking affine+acc back-to-back on the DVE queue
    dve_exp = [0, 2, 4, NT - 3, NT - 1]
    cand = [2 * int(round((3 + (k + 0.5) * (NT - 10) /
                           (N_DVE_EXP - 5)) / 2))
            for k in range(N_DVE_EXP - len(dve_exp))]
    for nt in cand:
        while nt // 2 in pe_red or nt // 2 >= NPAIR - 2 or nt in dve_exp:
            nt = (nt + 2) % NT
        dve_exp.append(nt)
    return set(dve_exp), pe_red


def build(tmix_pair, tile_sign, mc=MC, ncores=NCORES):
    """Build + compile the per-core program."""
    f32 = mybir.dt.float32
    bf16 = mybir.dt.bfloat16
    f8 = mybir.dt.float8e4
    i8 = mybir.dt.int8
    i16 = mybir.dt.int16
    EXP = mybir.ActivationFunctionType.Exp
    ALU = mybir.AluOpType
    DR = mybir.MatmulPerfMode.DoubleRow

    dve_exp, pe_red = _assignments(tmix_pair)

    nc = bacc.Bacc(
        "TRN2",
        target_bir_lowering=False,
        debug=False,
        enable_asserts=False,
        num_devices=ncores,
    )

    xt_d = nc.dram_tensor("xt", (D, mc), f8, kind="ExternalInput")
    bt_d = nc.dram_tensor("bt", (D, N), f8, kind="ExternalInput")
    wc_d = nc.dram_tensor("wc", (P, 2, NPAIR), f8, kind="ExternalInput")
    bias_d = nc.dram_tensor("bias_t", (P, NT), f32, kind="ExternalInput")
    c2_d = nc.dram_tensor("c2_t", (P, NT), f32, kind="ExternalInput")
    xx_d = nc.dram_tensor("xx", (1, mc), f32, kind="ExternalInput")
    bs_d = nc.dram_tensor("bs", (1, 1), f32, kind="ExternalInput")
    out_d = nc.dram_tensor("out", (1, mc), f32, kind="ExternalOutput")

    # x_train DMA groups: a fine-grained ladder so the first tiles land just
    # as the warmups finish, then 8-tile groups. Most bulk goes on the
    # gpsimd SWDGE queue (~2x the sync queue's throughput).
    groups = [(0, 2), (2, 4), (4, 8)]
    t0 = 8
    while t0 < NT:
        groups.append((t0, t0 + 8))
        t0 += 8

    with tile.TileContext(nc) as tc:
        with (
            tc.tile_pool(name="const", bufs=1) as const,
            tc.tile_pool(name="bt_pool", bufs=1) as bt_pool,
            tc.tile_pool(name="e8_pool", bufs=4) as e8_pool,
            tc.tile_pool(name="e16_pool", bufs=4) as e16_pool,
            tc.tile_pool(name="pt_pool", bufs=3, space="PSUM") as pt_pool,
            tc.tile_pool(name="ps_pool", bufs=1, space="PSUM") as ps_pool,
        ):
            xt_sb = const.tile([P, KT, mc], f8, name="xt_sb")
            wc_sb = const.tile([P, 2, NPAIR], f8, name="wc_sb")
            bias_sb = const.tile([P, NT], f32, name="bias_sb")
            c2_sb = const.tile([P, NT], f32, name="c2_sb")
            xx_sb = const.tile([1, mc], f32, name="xx_sb")
            bs_sb = const.tile([1, 1], f32, name="bs_sb")
            exmm = const.tile([1, mc], f32, name="exmm")
            fin = const.tile([1, mc], f32, name="fin")
            acc2 = const.tile([P, 2, mc], bf16, name="acc2")
            ones_sb = const.tile([P, 1], bf16, name="ones_sb")
            scrap = const.tile([P, MCH], bf16, name="scrap")

            bt_sb = {}
            for gi, (a, b) in enumerate(groups):
                bt_sb[gi] = bt_pool.tile(
                    [P, KT, (b - a) * P], f8, name=f"bt_sb{gi}"
                )

            def bt_slice(nt, p):
                for gi, (a, b) in enumerate(groups):
                    if a <= nt < b:
                        r = nt - a
                        return bt_sb[gi][:, 2 * p:2 * p + 2, r * P:(r + 1) * P]
                raise AssertionError

            # HAM warmup: near-dependency-free matmuls on scratch SBUF keep
            # the PE busy from the very start so the clock is at 2.4 GHz when
            # the first real matmul issues. Results discarded (real groups
            # restart with start=True).
            nc.vector.memset(scrap[:], 0.0)
            ps = [ps_pool.tile([1, MCH], f32, name=f"ps{j}") for j in range(MJ)]
            for _ in range(11):
                nc.tensor.matmul(ps[0][:], scrap[:, 0:1], scrap[:],
                                 start=True, stop=True)

            # --- DMA issue: the gpsimd SWDGE queue (faster) carries the
            # first tile group + the whole x slab + most of the bulk; the
            # sync HWDGE queue takes the second group, the tables, and late
            # groups; nothing on the scalar engine ---
            def load_bt(gi, eng):
                a, b = groups[gi]
                eng.dma_start(
                    bt_sb[gi][:],
                    bt_d[:, a * P:b * P].rearrange("(k p) n -> p k n", p=P),
                )

            load_bt(0, nc.gpsimd)
            nc.gpsimd.dma_start(
                xt_sb[:], xt_d[:].rearrange("(k p) m -> p k m", p=P)
            )
            nc.sync.dma_start(xx_sb[:], xx_d[:])
            load_bt(1, nc.sync)
            nc.sync.dma_start(bias_sb[:], bias_d[:])
            nc.sync.dma_start(c2_sb[:], c2_d[:])
            nc.sync.dma_start(wc_sb[:], wc_d[:])
            nc.sync.dma_start(bs_sb[:], bs_d[:])
            for gi in range(2, len(groups)):
                # sync only takes late groups (its demand deadlines are loose)
                eng = nc.sync if gi in (5, 7, 9) else nc.gpsimd
                load_bt(gi, eng)

            # the ones column for the accumulator fold (acc2 itself is
            # initialized by the first DVE-reduced pair's copy)
            nc.vector.memset(ones_sb[:], 1.0)

            # exp table load + per-m factor, as early as possible
            nc.scalar.activation(exmm[:], xx_sb[:], EXP)

            started = [False, False]   # ps[j] accumulation group opened?
            acc_init = True            # acc2 not yet written?
            pend = []
            acc_pend = []              # deferred DVE accumulates

            def emit_acc(ent):
                nonlocal acc_init
                e2, s0, s1 = ent
                if s0 == s1:
                    if acc_init and s0 >= 0:
                        nc.vector.tensor_copy(acc2[:], e2[:])
                    else:
                        op = ALU.add if s0 >= 0 else ALU.subtract
                        nc.vector.tensor_tensor(acc2[:], acc2[:], e2[:], op)
                else:
                    if acc_init:
                        nc.vector.memset(acc2[:], 0.0)
                    for h, s in ((0, s0), (1, s1)):
                        op = ALU.add if s >= 0 else ALU.subtract
                        nc.vector.tensor_tensor(
                            acc2[:, h, :], acc2[:, h, :], e2[:, h, :], op)
                acc_init = False

            def emit_red(ent):
                pe2, pi = ent
                for j in range(MJ):
                    nc.tensor.matmul(
                        ps[j][:],
                        wc_sb[:, :, pi:pi + 1],
                        pe2[:, :, j * MCH:(j + 1) * MCH],
                        start=not started[j],
                        stop=False,
                        perf_mode=DR,
                    )
                    started[j] = True

            for i in range(NPAIR):
                is_pe = i in pe_red
                if is_pe:
                    e2 = e8_pool.tile([P, 2, mc], f8, name="e8")
                    e2_int = e2[:].bitcast(i8)
                else:
                    e2 = e16_pool.tile([P, 2, mc], bf16, name="e16")
                    e2_int = e2[:].bitcast(i16)
                for h in range(2):
                    nt = 2 * i + h
                    pt = pt_pool.tile([P, mc], f32, name="pt")
                    for p in range(2):
                        for j in range(MJ):
                            nc.tensor.matmul(
                                pt[:, j * MCH:(j + 1) * MCH],
                                bt_slice(nt, p),
                                xt_sb[:, 2 * p:2 * p + 2,
                                      j * MCH:(j + 1) * MCH],
                                start=(p == 0),
                                stop=(p == 1),
                                perf_mode=DR,
                            )
                    if nt in dve_exp:
                        # Schraudolph exp: RNE+saturating int convert of the
                        # affine maps negatives to -0.0 and overflow to NaN,
                        # matching exp under/overflow
                        c1 = C1_8 if is_pe else C1_16
                        nc.vector.tensor_scalar(
                            e2_int[:, h, :], pt[:], c1, c2_sb[:, nt:nt + 1],
                            ALU.mult, ALU.add,
                        )
                    else:
                        nc.scalar.activation(
                            e2[:, h, :], pt[:], EXP,
                            bias=bias_sb[:, nt:nt + 1],
                        )
                if is_pe:
                    pend.append((e2, i))
                else:
                    acc_pend.append(
                        (i, (e2, tile_sign[2 * i], tile_sign[2 * i + 1])))
                while acc_pend and acc_pend[0][0] <= i:
                    emit_acc(acc_pend.pop(0)[1])
                while pend and pend[0][1] <= i - 2:
                    emit_red(pend.pop(0))

            for _, ent in acc_pend:
                emit_acc(ent)

            # final: per chunk, finish the remaining reductions, fold the DVE
            # accumulator into ps (closing the accumulation group), combine,
            # and ship — chunk 0's combine/DMA overlaps chunk 1's matmuls
            for j in range(MJ):
                for pe2, pi in pend:
                    nc.tensor.matmul(
                        ps[j][:],
                        wc_sb[:, :, pi:pi + 1],
                        pe2[:, :, j * MCH:(j + 1) * MCH],
                        start=not started[j],
                        stop=False,
                        perf_mode=DR,
                    )
                    started[j] = True
                for h in range(2):
                    nc.tensor.matmul(
                        ps[j][:], ones_sb[:],
                        acc2[:, h, j * MCH:(j + 1) * MCH],
                        start=not started[j], stop=(h == 1),
                    )
                    started[j] = True
                sl = slice(j * MCH, (j + 1) * MCH)
                nc.vector.tensor_mul(fin[:, sl], ps[j][:], exmm[:, sl])
                nc.vector.tensor_scalar_add(fin[:, sl], fin[:, sl], bs_sb[:])
                nc.sync.dma_start(out_d[:, sl], fin[:, sl])

    nc.compile()
    return nc


_CACHE = {}


def kernel(x, x_train, gamma, weight, bias):
    global LAST_RESULTS
    x = np.asarray(x, dtype=np.float32)
    x_train = np.asarray(x_train, dtype=np.float32)
    g = float(np.asarray(gamma).reshape(-1)[0])
    w = np.asarray(weight, dtype=np.float32).reshape(N)
    b = np.float32(np.asarray(bias).reshape(-1)[0])

    # sort training points: positive weights first, so n-tiles are sign-pure
    # (except one mixed tile, forced onto the matmul-reduction path)
    perm = np.argsort(w < 0, kind="stable")
    w = w[perm]
    x_train = x_train[perm]
    npos = int((w >= 0).sum())
    tmix = npos // P if npos % P != 0 else None
    tmix_pair = None if tmix is None else tmix // 2
    tile_sign = [1 if (t + 1) * P <= npos else (-1 if t * P >= npos else 0)
                 for t in range(NT)]
    dve_exp, pe_red = _assignments(tmix_pair)

    xx = np.einsum("md,md->m", x.astype(np.float64), x.astype(np.float64))
    yy = np.einsum("nd,nd->n", x_train.astype(np.float64),
                   x_train.astype(np.float64))

    xt_all = np.ascontiguousarray(x.T).astype(ml_dtypes.float8_e4m3)  # [D, M]
    bt = np.ascontiguousarray((2.0 * g) * x_train.T).astype(
        ml_dtypes.float8_e4m3)                                        # [D, N]
    # wc[p, r, i] = 256 * w[(2i + r)*128 + p]: DoubleRow reduction stationary.
    # w scaled by 256 keeps fp8e4m3 in its normal range; the final combine
    # absorbs 1/256 inside the exmm exp bias.
    wc = np.ascontiguousarray(
        (256.0 * w).reshape(NPAIR, 2, P).transpose(2, 1, 0)
    ).astype(ml_dtypes.float8_e4m3)

    bn = (-g * yy).astype(np.float64)                 # per-n exp bias
    lw = np.log(np.maximum(np.abs(256.0 * w), 1e-30)).astype(np.float64)
    bias_t = np.empty((NT, P), np.float32)            # ACT bias
    c2_t = np.empty((NT, P), np.float32)              # DVE affine add
    for t in range(NT):
        sl = slice(t * P, (t + 1) * P)
        is_pe = (t // 2) in pe_red
        bb = bn[sl] + (0.0 if is_pe else lw[sl])      # DVE-red: |256w| in exp
        bias_t[t] = bb.astype(np.float32)
        c1, c2 = (C1_8, C2_8) if is_pe else (C1_16, C2_16)
        c2_t[t] = (c1 * bb + c2).astype(np.float32)
    bias_t = np.ascontiguousarray(bias_t.T)           # [P, NT]
    c2_t = np.ascontiguousarray(c2_t.T)               # [P, NT]

    bs = np.full((1, 1), b, dtype=np.float32)
    xxn = (-g * xx - np.log(256.0)).astype(np.float32)

    in_maps = []
    for c in range(NCORES):
        sl = slice(c * MC, (c + 1) * MC)
        in_maps.append({
            "xt": np.ascontiguousarray(xt_all[:, sl]),
            "bt": bt,
            "wc": wc,
            "bias_t": bias_t,
            "c2_t": c2_t,
            "xx": xxn[sl].reshape(1, MC),
            "bs": bs,
        })

    key = (tmix_pair, tuple(tile_sign))
    if key not in _CACHE:
        _CACHE.clear()
        _CACHE[key] = build(tmix_pair, tile_sign)
    nc = _CACHE[key]
    res = run_bass_kernel_spmd(nc, in_maps, core_ids=list(range(NCORES)))
    LAST_RESULTS = res
    out = np.concatenate(
        [np.asarray(res.results[c]["out"], dtype=np.float32).reshape(MC)
         for c in range(NCORES)]
    )
    return out.reshape(M, 1)


# revision 35
# speedup vs baseline: 1.0132x; 1.0132x over previous
"""RBF-kernel SVM inference on 8 Trainium2 NeuronCores.

out[m] = sum_n w[n] * exp(-g * ||x[m] - xt[n]||^2) + b
       = exp(-g*||x[m]||^2) * sum_n w[n] * exp(2g*x[m].xt[n] - g*||xt[n]||^2) + b

Sharding: rows of x split across 8 cores (1024 each); x_train / weight
replicated. Per core: a [8192, 1024] kernel slab via fp8 DoubleRow matmuls
(K=512 contraction, n on partitions).

The exp and the n-reduction are split across three engines to balance load:
  - PE: main matmuls (fixed cost) + DoubleRow reduction matmuls for a few
    "PE-red" pairs (fp8 e-planes, weights in the stationary) + a final bf16
    ones-reduction of the DVE accumulator.
  - ACT: exp for most tiles; the per-partition bias carries -g*||xt||^2, plus
    ln(256|w|) for DVE-reduced tiles (weights folded into the exp).
  - DVE: Schraudolph-style exp for some tiles (one tensor_scalar: affine into
    bf16/fp8 bit space with RNE saturating int convert ~= 2^((b-bias)/mant);
    negative saturation yields -0.0 = exp underflow), and accumulation of the
    DVE-reduced pairs' bf16 e-planes into a bf16 accumulator (training points
    pre-sorted by sign(w) so tiles are sign-pure and +/- becomes add/subtract).

Startup: the x slab and lead x_train groups split across the sync HWDGE and
gpsimd SWDGE queues (nothing on the scalar engine, which the exp saturates),
and warmup matmuls on scratch so the PE clock (HAM) is at 2.4 GHz and the
DMA data is resident when the first real matmul lands.
"""

import os
import sys

for _p in ("/opt/trn_rl_repo", "/root/.axon_site/_ro/trn_rl_repo"):
    if os.path.isdir(_p) and _p not in sys.path:
        sys.path.append(_p)

import numpy as np
import ml_dtypes

import concourse.bass as bass
import concourse.mybir as mybir
import concourse.tile as tile
from concourse import bacc
from concourse.bass_utils import run_bass_kernel_spmd

M, N, D = 8192, 8192, 512
NCORES = 8
MC = M // NCORES          # rows of x per core
P = 128
KT = D // P               # K tiles (4)
NT = N // P               # n tiles (64)
NPAIR = NT // 2           # n-tile pairs (32)
MCH = 512                 # PSUM free-dim chunk (one f32 bank)
MJ = MC // MCH            # m chunks (2)

C1_8 = 8.0 / float(np.log(2.0))      # fp8e4m3 bits per ln unit
C2_8 = 56.0                          # fp8e4m3 exponent bias offset
C1_16 = 128.0 / float(np.log(2.0))   # bf16 bits per ln unit
C2_16 = 16256.0                      # bf16 exponent bias offset (127*128)

LAST_RESULTS = None  # BassKernelResults of the most recent run (for test.py)

N_DVE_EXP = 15


def _assignments(tmix_pair):
    # PE-red pairs: the mixed-sign pair + the last two pairs (so the tail
    # doesn't wait on the DVE accumulator)
    pe_red = {NPAIR - 2, NPAIR - 1}
    if tmix_pair is not None:
        pe_red.add(tmix_pair)
    # DVE-exp tiles: alternate with ACT over the first tiles (pipeline fill),
    # one tile of each of the last two pairs (parallel tail), rest spread
    dve_exp = [0, 2, 4, NT - 3, NT - 1]
    cand = [int(round(6 + (k + 0.5) * (NT - 10) / (N_DVE_EXP - 5)))
            for k in range(N_DVE_EXP - len(dve_exp))]
    for nt in cand:
        while nt // 2 in pe_red or nt // 2 >= NPAIR - 2 or nt in dve_exp:
            nt = (nt + 1) % NT
        dve_exp.append(nt)
    return set(dve_exp), pe_red


def build(tmix_pair, tile_sign, mc=MC, ncores=NCORES):
    """Build + compile the per-core program."""
    f32 = mybir.dt.float32
    bf16 = mybir.dt.bfloat16
    f8 = mybir.dt.float8e4
    i8 = mybir.dt.int8
    i16 = mybir.dt.int16
    EXP = mybir.ActivationFunctionType.Exp
    ALU = mybir.AluOpType
    DR = mybir.MatmulPerfMode.DoubleRow

    dve_exp, pe_red = _assignments(tmix_pair)

    nc = bacc.Bacc(
        "TRN2",
        target_bir_lowering=False,
        debug=False,
        enable_asserts=False,
        num_devices=ncores,
    )

    xt_d = nc.dram_tensor("xt", (D, mc), f8, kind="ExternalInput")
    bt_d = nc.dram_tensor("bt", (D, N), f8, kind="ExternalInput")
    wc_d = nc.dram_tensor("wc", (P, 2, NPAIR), f8, kind="ExternalInput")
    bias_d = nc.dram_tensor("bias_t", (P, NT), f32, kind="ExternalInput")
    c2_d = nc.dram_tensor("c2_t", (P, NT), f32, kind="ExternalInput")
    xx_d = nc.dram_tensor("xx", (1, mc), f32, kind="ExternalInput")
    bs_d = nc.dram_tensor("bs", (1, 1), f32, kind="ExternalInput")
    out_d = nc.dram_tensor("out", (1, mc), f32, kind="ExternalOutput")

    # x_train DMA groups: a fine-grained ladder so the first tiles land just
    # as the warmups finish, then 8-tile groups. Most bulk goes on the
    # gpsimd SWDGE queue (~2x the sync queue's throughput).
    groups = [(0, 2), (2, 4), (4, 8)]
    t0 = 8
    while t0 < NT:
        groups.append((t0, t0 + 8))
        t0 += 8

    with tile.TileContext(nc) as tc:
        with (
            tc.tile_pool(name="const", bufs=1) as const,
            tc.tile_pool(name="bt_pool", bufs=1) as bt_pool,
            tc.tile_pool(name="e8_pool", bufs=4) as e8_pool,
            tc.tile_pool(name="e16_pool", bufs=4) as e16_pool,
            tc.tile_pool(name="pt_pool", bufs=3, space="PSUM") as pt_pool,
            tc.tile_pool(name="ps_pool", bufs=1, space="PSUM") as ps_pool,
        ):
            xt_sb = const.tile([P, KT, mc], f8, name="xt_sb")
            wc_sb = const.tile([P, 2, NPAIR], f8, name="wc_sb")
            bias_sb = const.tile([P, NT], f32, name="bias_sb")
            c2_sb = const.tile([P, NT], f32, name="c2_sb")
            xx_sb = const.tile([1, mc], f32, name="xx_sb")
            bs_sb = const.tile([1, 1], f32, name="bs_sb")
            exmm = const.tile([1, mc], f32, name="exmm")
            fin = const.tile([1, mc], f32, name="fin")
            acc2 = const.tile([P, 2, mc], bf16, name="acc2")
            ones_sb = const.tile([P, 1], bf16, name="ones_sb")
            scrap = const.tile([P, MCH], bf16, name="scrap")

            bt_sb = {}
            for gi, (a, b) in enumerate(groups):
                bt_sb[gi] = bt_pool.tile(
                    [P, KT, (b - a) * P], f8, name=f"bt_sb{gi}"
                )

            def bt_slice(nt, p):
                for gi, (a, b) in enumerate(groups):
                    if a <= nt < b:
                        r = nt - a
                        return bt_sb[gi][:, 2 * p:2 * p + 2, r * P:(r + 1) * P]
                raise AssertionError

            # HAM warmup: near-dependency-free matmuls on scratch SBUF keep
            # the PE busy from the very start so the clock is at 2.4 GHz when
            # the first real matmul issues. Results discarded (real groups
            # restart with start=True).
            nc.vector.memset(scrap[:], 0.0)
            ps = [ps_pool.tile([1, MCH], f32, name=f"ps{j}") for j in range(MJ)]
            for _ in range(13):
                nc.tensor.matmul(ps[0][:], scrap[:, 0:1], scrap[:],
                                 start=True, stop=True)

            # --- DMA issue: the x slab is split across both queues; the
            # gpsimd SWDGE queue (faster) carries the lead x_train groups and
            # most of the bulk; nothing on the scalar engine ---
            nc.sync.dma_start(xx_sb[:], xx_d[:])
            nc.sync.dma_start(
                xt_sb[:, 0:2, :],
                xt_d[0:2 * P, :].rearrange("(k p) m -> p k m", p=P),
            )
            nc.gpsimd.dma_start(
                xt_sb[:, 2:4, :],
                xt_d[2 * P:4 * P, :].rearrange("(k p) m -> p k m", p=P),
            )

            def load_bt(gi, eng):
                a, b = groups[gi]
                eng.dma_start(
                    bt_sb[gi][:],
                    bt_d[:, a * P:b * P].rearrange("(k p) n -> p k n", p=P),
                )

            load_bt(0, nc.gpsimd)
            load_bt(1, nc.gpsimd)
            nc.sync.dma_start(bias_sb[:], bias_d[:])
            nc.sync.dma_start(c2_sb[:], c2_d[:])
            nc.sync.dma_start(wc_sb[:], wc_d[:])
            nc.sync.dma_start(bs_sb[:], bs_d[:])
            for gi in range(2, len(groups)):
                # sync only takes late groups (its demand deadlines are loose)
                eng = nc.sync if gi in (6, 8, 10) else nc.gpsimd
                load_bt(gi, eng)

            # the ones column for the accumulator fold (acc2 itself is
            # initialized by the first DVE-reduced pair's copy)
            nc.vector.memset(ones_sb[:], 1.0)

            # exp table load + per-m factor, as early as possible
            nc.scalar.activation(exmm[:], xx_sb[:], EXP)

            started = [False, False]   # ps[j] accumulation group opened?
            acc_init = True            # acc2 not yet written?
            pend = []
            acc_pend = []              # deferred DVE accumulates

            def emit_acc(ent):
                nonlocal acc_init
                e2, s0, s1 = ent
                if s0 == s1:
                    if acc_init and s0 >= 0:
                        nc.vector.tensor_copy(acc2[:], e2[:])
                    else:
                        op = ALU.add if s0 >= 0 else ALU.subtract
                        nc.vector.tensor_tensor(acc2[:], acc2[:], e2[:], op)
                else:
                    if acc_init:
                        nc.vector.memset(acc2[:], 0.0)
                    for h, s in ((0, s0), (1, s1)):
                        op = ALU.add if s >= 0 else ALU.subtract
                        nc.vector.tensor_tensor(
                            acc2[:, h, :], acc2[:, h, :], e2[:, h, :], op)
                acc_init = False

            def emit_red(ent):
                pe2, pi = ent
                for j in range(MJ):
                    nc.tensor.matmul(
                        ps[j][:],
                        wc_sb[:, :, pi:pi + 1],
                        pe2[:, :, j * MCH:(j + 1) * MCH],
                        start=not started[j],
                        stop=False,
                        perf_mode=DR,
                    )
                    started[j] = True

            for i in range(NPAIR):
                is_pe = i in pe_red
                if is_pe:
                    e2 = e8_pool.tile([P, 2, mc], f8, name="e8")
                    e2_int = e2[:].bitcast(i8)
                else:
                    e2 = e16_pool.tile([P, 2, mc], bf16, name="e16")
                    e2_int = e2[:].bitcast(i16)
                for h in range(2):
                    nt = 2 * i + h
                    pt = pt_pool.tile([P, mc], f32, name="pt")
                    for p in range(2):
                        for j in range(MJ):
                            nc.tensor.matmul(
                                pt[:, j * MCH:(j + 1) * MCH],
                                bt_slice(nt, p),
                                xt_sb[:, 2 * p:2 * p + 2,
                                      j * MCH:(j + 1) * MCH],
                                start=(p == 0),
                                stop=(p == 1),
                                perf_mode=DR,
                            )
                    if nt in dve_exp:
                        # Schraudolph exp: RNE+saturating int convert of the
                        # affine maps negatives to -0.0 and overflow to NaN,
                        # matching exp under/overflow
                        c1 = C1_8 if is_pe else C1_16
                        nc.vector.tensor_scalar(
                            e2_int[:, h, :], pt[:], c1, c2_sb[:, nt:nt + 1],
                            ALU.mult, ALU.add,
                        )
                    else:
                        nc.scalar.activation(
                            e2[:, h, :], pt[:], EXP,
                            bias=bias_sb[:, nt:nt + 1],
                        )
                if is_pe:
                    pend.append((e2, i))
                else:
                    acc_pend.append(
                        (i, (e2, tile_sign[2 * i], tile_sign[2 * i + 1])))
                while acc_pend and acc_pend[0][0] <= i:
                    emit_acc(acc_pend.pop(0)[1])
                while pend and pend[0][1] <= i - 2:
                    emit_red(pend.pop(0))

            for _, ent in acc_pend:
                emit_acc(ent)

            # final: per chunk, finish the remaining reductions, fold the DVE
            # accumulator into ps (closing the accumulation group), combine,
            # and ship — chunk 0's combine/DMA overlaps chunk 1's matmuls
            for j in range(MJ):
                for pe2, pi in pend:
                    nc.tensor.matmul(
                        ps[j][:],
                        wc_sb[:, :, pi:pi + 1],
                        pe2[:, :, j * MCH:(j + 1) * MCH],
                        start=not started[j],
                        stop=False,
                        perf_mode=DR,
                    )
                    started[j] = True
                for h in range(2):
                    nc.tensor.matmul(
                        ps[j][:], ones_sb[:],
                        acc2[:, h, j * MCH:(j + 1) * MCH],
                        start=not started[j], stop=(h == 1),
                    )
                    started[j] = True
                sl = slice(j * MCH, (j + 1) * MCH)
                nc.vector.tensor_mul(fin[:, sl], ps[j][:], exmm[:, sl])
                nc.vector.tensor_scalar_add(fin[:, sl], fin[:, sl], bs_sb[:])
                nc.sync.dma_start(out_d[:, sl], fin[:, sl])

    nc.compile()
    return nc


_CACHE = {}


def kernel(x, x_train, gamma, weight, bias):
    global LAST_RESULTS
    x = np.asarray(x, dtype=np.float32)
    x_train = np.asarray(x_train, dtype=np.float32)
    g = float(np.asarray(gamma).reshape(-1)[0])
    w = np.asarray(weight, dtype=np.float32).reshape(N)
    b = np.float32(np.asarray(bias).reshape(-1)[0])

    # sort training points: positive weights first, so n-tiles are sign-pure
    # (except one mixed tile, forced onto the matmul-reduction path)
    perm = np.argsort(w < 0, kind="stable")
    w = w[perm]
    x_train = x_train[perm]
    npos = int((w >= 0).sum())
    tmix = npos // P if npos % P != 0 else None
    tmix_pair = None if tmix is None else tmix // 2
    tile_sign = [1 if (t + 1) * P <= npos else (-1 if t * P >= npos else 0)
                 for t in range(NT)]
    dve_exp, pe_red = _assignments(tmix_pair)

    xx = np.einsum("md,md->m", x.astype(np.float64), x.astype(np.float64))
    yy = np.einsum("nd,nd->n", x_train.astype(np.float64),
                   x_train.astype(np.float64))

    xt_all = np.ascontiguousarray(x.T).astype(ml_dtypes.float8_e4m3)  # [D, M]
    bt = np.ascontiguousarray((2.0 * g) * x_train.T).astype(
        ml_dtypes.float8_e4m3)                                        # [D, N]
    # wc[p, r, i] = 256 * w[(2i + r)*128 + p]: DoubleRow reduction stationary.
    # w scaled by 256 keeps fp8e4m3 in its normal range; the final combine
    # absorbs 1/256 inside the exmm exp bias.
    wc = np.ascontiguousarray(
        (256.0 * w).reshape(NPAIR, 2, P).transpose(2, 1, 0)
    ).astype(ml_dtypes.float8_e4m3)

    bn = (-g * yy).astype(np.float64)                 # per-n exp bias
    lw = np.log(np.maximum(np.abs(256.0 * w), 1e-30)).astype(np.float64)
    bias_t = np.empty((NT, P), np.float32)            # ACT bias
    c2_t = np.empty((NT, P), np.float32)              # DVE affine add
    for t in range(NT):
        sl = slice(t * P, (t + 1) * P)
        is_pe = (t // 2) in pe_red
        bb = bn[sl] + (0.0 if is_pe else lw[sl])      # DVE-red: |256w| in exp
        bias_t[t] = bb.astype(np.float32)
        c1, c2 = (C1_8, C2_8) if is_pe else (C1_16, C2_16)
        c2_t[t] = (c1 * bb + c2).astype(np.float32)
    bias_t = np.ascontiguousarray(bias_t.T)           # [P, NT]
    c2_t = np.ascontiguousarray(c2_t.T)               # [P, NT]

    bs = np.full((1, 1), b, dtype=np.float32)
    xxn = (-g * xx - np.log(256.0)).astype(np.float32)

    in_maps = []
    for c in range(NCORES):
        sl = slice(c * MC, (c + 1) * MC)
        in_maps.append({
            "xt": np.ascontiguousarray(xt_all[:, sl]),
            "bt": bt,
            "wc": wc,
            "bias_t": bias_t,
            "c2_t": c2_t,
            "xx": xxn[sl].reshape(1, MC),
            "bs": bs,
        })

    key = (tmix_pair, tuple(tile_sign))
    if key not in _CACHE:
        _CACHE.clear()
        _CACHE[key] = build(tmix_pair, tile_sign)
    nc = _CACHE[key]
    res = run_bass_kernel_spmd(nc, in_maps, core_ids=list(range(NCORES)))
    LAST_RESULTS = res
    out = np.concatenate(
        [np.asarray(res.results[c]["out"], dtype=np.float32).reshape(MC)
         for c in range(NCORES)]
    )
    return out.reshape(M, 1)


# revision 37
# speedup vs baseline: 1.0199x; 1.0066x over previous
"""RBF-kernel SVM inference on 8 Trainium2 NeuronCores.

out[m] = sum_n w[n] * exp(-g * ||x[m] - xt[n]||^2) + b
       = exp(-g*||x[m]||^2) * sum_n w[n] * exp(2g*x[m].xt[n] - g*||xt[n]||^2) + b

Sharding: rows of x split across 8 cores (1024 each); x_train / weight
replicated. Per core: a [8192, 1024] kernel slab via fp8 DoubleRow matmuls
(K=512 contraction, n on partitions).

The exp and the n-reduction are split across three engines to balance load:
  - PE: main matmuls (fixed cost) + DoubleRow reduction matmuls for a few
    "PE-red" pairs (fp8 e-planes, weights in the stationary) + a final bf16
    ones-reduction of the DVE accumulator.
  - ACT: exp for most tiles; the per-partition bias carries -g*||xt||^2, plus
    ln(256|w|) for DVE-reduced tiles (weights folded into the exp).
  - DVE: Schraudolph-style exp for some tiles (one tensor_scalar: affine into
    bf16/fp8 bit space with RNE saturating int convert ~= 2^((b-bias)/mant);
    negative saturation yields -0.0 = exp underflow), and accumulation of the
    DVE-reduced pairs' bf16 e-planes into a bf16 accumulator (training points
    pre-sorted by sign(w) so tiles are sign-pure and +/- becomes add/subtract).

Startup: the x slab and lead x_train groups split across the sync HWDGE and
gpsimd SWDGE queues (nothing on the scalar engine, which the exp saturates),
and warmup matmuls on scratch so the PE clock (HAM) is at 2.4 GHz and the
DMA data is resident when the first real matmul lands.
"""

import os
import sys

for _p in ("/opt/trn_rl_repo", "/root/.axon_site/_ro/trn_rl_repo"):
    if os.path.isdir(_p) and _p not in sys.path:
        sys.path.append(_p)

import numpy as np
import ml_dtypes

import concourse.bass as bass
import concourse.mybir as mybir
import concourse.tile as tile
from concourse import bacc
from concourse.bass_utils import run_bass_kernel_spmd

M, N, D = 8192, 8192, 512
NCORES = 8
MC = M // NCORES          # rows of x per core
P = 128
KT = D // P               # K tiles (4)
NT = N // P               # n tiles (64)
NPAIR = NT // 2           # n-tile pairs (32)
MCH = 512                 # PSUM free-dim chunk (one f32 bank)
MJ = MC // MCH            # m chunks (2)

C1_8 = 8.0 / float(np.log(2.0))      # fp8e4m3 bits per ln unit
C2_8 = 56.0                          # fp8e4m3 exponent bias offset
C1_16 = 128.0 / float(np.log(2.0))   # bf16 bits per ln unit
C2_16 = 16256.0                      # bf16 exponent bias offset (127*128)

LAST_RESULTS = None  # BassKernelResults of the most recent run (for test.py)

N_DVE_EXP = 15


def _assignments(tmix_pair):
    # PE-red pairs: the mixed-sign pair + the last two pairs (so the tail
    # doesn't wait on the DVE accumulator)
    pe_red = {NPAIR - 2, NPAIR - 1}
    if tmix_pair is not None:
        pe_red.add(tmix_pair)
    # DVE-exp tiles: alternate with ACT over the first tiles (pipeline fill),
    # one tile of each of the last two pairs (parallel tail), rest spread
    dve_exp = [0, 2, 4, NT - 3, NT - 1]
    cand = [int(round(6 + (k + 0.5) * (NT - 10) / (N_DVE_EXP - 5)))
            for k in range(N_DVE_EXP - len(dve_exp))]
    for nt in cand:
        while nt // 2 in pe_red or nt // 2 >= NPAIR - 2 or nt in dve_exp:
            nt = (nt + 1) % NT
        dve_exp.append(nt)
    return set(dve_exp), pe_red


def build(tmix_pair, tile_sign, mc=MC, ncores=NCORES):
    """Build + compile the per-core program."""
    f32 = mybir.dt.float32
    bf16 = mybir.dt.bfloat16
    f8 = mybir.dt.float8e4
    i8 = mybir.dt.int8
    i16 = mybir.dt.int16
    EXP = mybir.ActivationFunctionType.Exp
    ALU = mybir.AluOpType
    DR = mybir.MatmulPerfMode.DoubleRow

    dve_exp, pe_red = _assignments(tmix_pair)

    nc = bacc.Bacc(
        "TRN2",
        target_bir_lowering=False,
        debug=False,
        enable_asserts=False,
        num_devices=ncores,
    )

    xt_d = nc.dram_tensor("xt", (D, mc), f8, kind="ExternalInput")
    bt_d = nc.dram_tensor("bt", (D, N), f8, kind="ExternalInput")
    wc_d = nc.dram_tensor("wc", (P, 2, NPAIR), f8, kind="ExternalInput")
    bias_d = nc.dram_tensor("bias_t", (P, NT), f32, kind="ExternalInput")
    c2_d = nc.dram_tensor("c2_t", (P, NT), f32, kind="ExternalInput")
    xx_d = nc.dram_tensor("xx", (1, mc), f32, kind="ExternalInput")
    bs_d = nc.dram_tensor("bs", (1, 1), f32, kind="ExternalInput")
    out_d = nc.dram_tensor("out", (1, mc), f32, kind="ExternalOutput")

    # x_train DMA groups: a fine-grained ladder so the first tiles land just
    # as the warmups finish, then 8-tile groups. Most bulk goes on the
    # gpsimd SWDGE queue (~2x the sync queue's throughput).
    groups = [(0, 2), (2, 4), (4, 8)]
    t0 = 8
    while t0 < NT:
        groups.append((t0, t0 + 8))
        t0 += 8

    with tile.TileContext(nc) as tc:
        with (
            tc.tile_pool(name="const", bufs=1) as const,
            tc.tile_pool(name="bt_pool", bufs=1) as bt_pool,
            tc.tile_pool(name="e8_pool", bufs=4) as e8_pool,
            tc.tile_pool(name="e16_pool", bufs=4) as e16_pool,
            tc.tile_pool(name="pt_pool", bufs=3, space="PSUM") as pt_pool,
            tc.tile_pool(name="ps_pool", bufs=1, space="PSUM") as ps_pool,
        ):
            xt_sb = const.tile([P, KT, mc], f8, name="xt_sb")
            wc_sb = const.tile([P, 2, NPAIR], f8, name="wc_sb")
            bias_sb = const.tile([P, NT], f32, name="bias_sb")
            c2_sb = const.tile([P, NT], f32, name="c2_sb")
            xx_sb = const.tile([1, mc], f32, name="xx_sb")
            bs_sb = const.tile([1, 1], f32, name="bs_sb")
            exmm = const.tile([1, mc], f32, name="exmm")
            tld = const.tile([1, 1], f32, name="tld")
            fin = const.tile([1, mc], f32, name="fin")
            acc2 = const.tile([P, 2, mc], bf16, name="acc2")
            ones_sb = const.tile([P, 1], bf16, name="ones_sb")
            scrap = const.tile([P, MCH], bf16, name="scrap")

            bt_sb = {}
            for gi, (a, b) in enumerate(groups):
                bt_sb[gi] = bt_pool.tile(
                    [P, KT, (b - a) * P], f8, name=f"bt_sb{gi}"
                )

            def bt_slice(nt, p):
                for gi, (a, b) in enumerate(groups):
                    if a <= nt < b:
                        r = nt - a
                        return bt_sb[gi][:, 2 * p:2 * p + 2, r * P:(r + 1) * P]
                raise AssertionError

            # HAM warmup: near-dependency-free matmuls on scratch SBUF keep
            # the PE busy from the very start so the clock is at 2.4 GHz when
            # the first real matmul issues. Results discarded (real groups
            # restart with start=True).
            nc.vector.memset(scrap[:], 0.0)
            ps = [ps_pool.tile([1, MCH], f32, name=f"ps{j}") for j in range(MJ)]
            for _ in range(13):
                nc.tensor.matmul(ps[0][:], scrap[:, 0:1], scrap[:],
                                 start=True, stop=True)

            # --- DMA issue: the x slab is split across both queues; the
            # gpsimd SWDGE queue (faster) carries the lead x_train groups and
            # most of the bulk; nothing on the scalar engine ---
            nc.sync.dma_start(xx_sb[:], xx_d[:])
            nc.sync.dma_start(
                xt_sb[:, 0:2, :],
                xt_d[0:2 * P, :].rearrange("(k p) m -> p k m", p=P),
            )
            nc.gpsimd.dma_start(
                xt_sb[:, 2:4, :],
                xt_d[2 * P:4 * P, :].rearrange("(k p) m -> p k m", p=P),
            )

            def load_bt(gi, eng):
                a, b = groups[gi]
                eng.dma_start(
                    bt_sb[gi][:],
                    bt_d[:, a * P:b * P].rearrange("(k p) n -> p k n", p=P),
                )

            load_bt(0, nc.gpsimd)
            load_bt(1, nc.gpsimd)
            nc.sync.dma_start(bias_sb[:], bias_d[:])
            nc.sync.dma_start(c2_sb[:], c2_d[:])
            nc.sync.dma_start(wc_sb[:], wc_d[:])
            nc.sync.dma_start(bs_sb[:], bs_d[:])
            for gi in range(2, len(groups)):
                # sync only takes late groups (its demand deadlines are loose)
                eng = nc.sync if gi in (6, 8, 10) else nc.gpsimd
                load_bt(gi, eng)

            # the ones column for the accumulator fold (acc2 itself is
            # initialized by the first DVE-reduced pair's copy)
            nc.vector.memset(ones_sb[:], 1.0)

            # trigger the ~2.7us exp-table load with a tiny dummy activation
            # that depends only on the memset above (not on any DMA), so ACT
            # enters the pipeline early; then the per-m factor once xx lands
            nc.scalar.activation(tld[:], ones_sb[0:1, 0:1], EXP)
            nc.scalar.activation(exmm[:], xx_sb[:], EXP)

            started = [False, False]   # ps[j] accumulation group opened?
            acc_init = True            # acc2 not yet written?
            pend = []
            acc_pend = []              # deferred DVE accumulates

            def emit_acc(ent):
                nonlocal acc_init
                e2, s0, s1 = ent
                if s0 == s1:
                    if acc_init and s0 >= 0:
                        nc.vector.tensor_copy(acc2[:], e2[:])
                    else:
                        op = ALU.add if s0 >= 0 else ALU.subtract
                        nc.vector.tensor_tensor(acc2[:], acc2[:], e2[:], op)
                else:
                    if acc_init:
                        nc.vector.memset(acc2[:], 0.0)
                    for h, s in ((0, s0), (1, s1)):
                        op = ALU.add if s >= 0 else ALU.subtract
                        nc.vector.tensor_tensor(
                            acc2[:, h, :], acc2[:, h, :], e2[:, h, :], op)
                acc_init = False

            def emit_red(ent):
                pe2, pi = ent
                for j in range(MJ):
                    nc.tensor.matmul(
                        ps[j][:],
                        wc_sb[:, :, pi:pi + 1],
                        pe2[:, :, j * MCH:(j + 1) * MCH],
                        start=not started[j],
                        stop=False,
                        perf_mode=DR,
                    )
                    started[j] = True

            for i in range(NPAIR):
                is_pe = i in pe_red
                if is_pe:
                    e2 = e8_pool.tile([P, 2, mc], f8, name="e8")
                    e2_int = e2[:].bitcast(i8)
                else:
                    e2 = e16_pool.tile([P, 2, mc], bf16, name="e16")
                    e2_int = e2[:].bitcast(i16)
                for h in range(2):
                    nt = 2 * i + h
                    pt = pt_pool.tile([P, mc], f32, name="pt")
                    for p in range(2):
                        for j in range(MJ):
                            nc.tensor.matmul(
                                pt[:, j * MCH:(j + 1) * MCH],
                                bt_slice(nt, p),
                                xt_sb[:, 2 * p:2 * p + 2,
                                      j * MCH:(j + 1) * MCH],
                                start=(p == 0),
                                stop=(p == 1),
                                perf_mode=DR,
                            )
                    if nt in dve_exp:
                        # Schraudolph exp: RNE+saturating int convert of the
                        # affine maps negatives to -0.0 and overflow to NaN,
                        # matching exp under/overflow
                        c1 = C1_8 if is_pe else C1_16
                        nc.vector.tensor_scalar(
                            e2_int[:, h, :], pt[:], c1, c2_sb[:, nt:nt + 1],
                            ALU.mult, ALU.add,
                        )
                    else:
                        nc.scalar.activation(
                            e2[:, h, :], pt[:], EXP,
                            bias=bias_sb[:, nt:nt + 1],
                        )
                if is_pe:
                    pend.append((e2, i))
                else:
                    acc_pend.append(
                        (i, (e2, tile_sign[2 * i], tile_sign[2 * i + 1])))
                while acc_pend and acc_pend[0][0] <= i:
                    emit_acc(acc_pend.pop(0)[1])
                while pend and pend[0][1] <= i - 2:
                    emit_red(pend.pop(0))

            for _, ent in acc_pend:
                emit_acc(ent)

            # final: per chunk, finish the remaining reductions, fold the DVE
            # accumulator into ps (closing the accumulation group), combine,
            # and ship — chunk 0's combine/DMA overlaps chunk 1's matmuls
            for j in range(MJ):
                for pe2, pi in pend:
                    nc.tensor.matmul(
                        ps[j][:],
                        wc_sb[:, :, pi:pi + 1],
                        pe2[:, :, j * MCH:(j + 1) * MCH],
                        start=not started[j],
                        stop=False,
                        perf_mode=DR,
                    )
                    started[j] = True
                for h in range(2):
                    nc.tensor.matmul(
                        ps[j][:], ones_sb[:],
                        acc2[:, h, j * MCH:(j + 1) * MCH],
                        start=not started[j], stop=(h == 1),
                    )
                    started[j] = True
                sl = slice(j * MCH, (j + 1) * MCH)
                nc.vector.tensor_mul(fin[:, sl], ps[j][:], exmm[:, sl])
                nc.vector.tensor_scalar_add(fin[:, sl], fin[:, sl], bs_sb[:])
                nc.sync.dma_start(out_d[:, sl], fin[:, sl])

    nc.compile()
    return nc


_CACHE = {}


def kernel(x, x_train, gamma, weight, bias):
    global LAST_RESULTS
    x = np.asarray(x, dtype=np.float32)
    x_train = np.asarray(x_train, dtype=np.float32)
    g = float(np.asarray(gamma).reshape(-1)[0])
    w = np.asarray(weight, dtype=np.float32).reshape(N)
    b = np.float32(np.asarray(bias).reshape(-1)[0])

    # sort training points: positive weights first, so n-tiles are sign-pure
    # (except one mixed tile, forced onto the matmul-reduction path)
    perm = np.argsort(w < 0, kind="stable")
    w = w[perm]
    x_train = x_train[perm]
    npos = int((w >= 0).sum())
    tmix = npos // P if npos % P != 0 else None
    tmix_pair = None if tmix is None else tmix // 2
    tile_sign = [1 if (t + 1) * P <= npos else (-1 if t * P >= npos else 0)
                 for t in range(NT)]
    dve_exp, pe_red = _assignments(tmix_pair)

    xx = np.einsum("md,md->m", x.astype(np.float64), x.astype(np.float64))
    yy = np.einsum("nd,nd->n", x_train.astype(np.float64),
                   x_train.astype(np.float64))

    xt_all = np.ascontiguousarray(x.T).astype(ml_dtypes.float8_e4m3)  # [D, M]
    bt = np.ascontiguousarray((2.0 * g) * x_train.T).astype(
        ml_dtypes.float8_e4m3)                                        # [D, N]
    # wc[p, r, i] = 256 * w[(2i + r)*128 + p]: DoubleRow reduction stationary.
    # w scaled by 256 keeps fp8e4m3 in its normal range; the final combine
    # absorbs 1/256 inside the exmm exp bias.
    wc = np.ascontiguousarray(
        (256.0 * w).reshape(NPAIR, 2, P).transpose(2, 1, 0)
    ).astype(ml_dtypes.float8_e4m3)

    bn = (-g * yy).astype(np.float64)                 # per-n exp bias
    lw = np.log(np.maximum(np.abs(256.0 * w), 1e-30)).astype(np.float64)
    bias_t = np.empty((NT, P), np.float32)            # ACT bias
    c2_t = np.empty((NT, P), np.float32)              # DVE affine add
    for t in range(NT):
        sl = slice(t * P, (t + 1) * P)
        is_pe = (t // 2) in pe_red
        bb = bn[sl] + (0.0 if is_pe else lw[sl])      # DVE-red: |256w| in exp
        bias_t[t] = bb.astype(np.float32)
        c1, c2 = (C1_8, C2_8) if is_pe else (C1_16, C2_16)
        c2_t[t] = (c1 * bb + c2).astype(np.float32)
    bias_t = np.ascontiguousarray(bias_t.T)           # [P, NT]
    c2_t = np.ascontiguousarray(c2_t.T)               # [P, NT]

    bs = np.full((1, 1), b, dtype=np.float32)
    xxn = (-g * xx - np.log(256.0)).astype(np.float32)

    in_maps = []
    for c in range(NCORES):
        sl = slice(c * MC, (c + 1) * MC)
        in_maps.append({
            "xt": np.ascontiguousarray(xt_all[:, sl]),
            "bt": bt,
            "wc": wc,
            "bias_t": bias_t,
            "c2_t": c2_t,
            "xx": xxn[sl].reshape(1, MC),
            "bs": bs,
        })

    key = (tmix_pair, tuple(tile_sign))
    if key not in _CACHE:
        _CACHE.clear()
        _CACHE[key] = build(tmix_pair, tile_sign)
    nc = _CACHE[key]
    res = run_bass_kernel_spmd(nc, in_maps, core_ids=list(range(NCORES)))
    LAST_RESULTS = res
    out = np.concatenate(
        [np.asarray(res.results[c]["out"], dtype=np.float32).reshape(MC)
         for c in range(NCORES)]
    )
    return out.reshape(M, 1)
